# revision 12
# baseline (speedup 1.0000x reference)
"""BinaryLinear kernel for 8 Trainium2 NeuronCores.

Computes out = x @ sign(W).T + bias for x [8, 2048, 4096], W [4096, 4096],
bias [4096], all float32.

Strategy: data-parallel over the batch dim — core b handles x[b] ([2048
tokens, 4096 in]) with the full (binarized) weight matrix.

Per-core device kernel (Tile framework) — MIXED-PRECISION contraction:
  - The 32 contraction k-tiles are split KB=16 in bf16 + 8 fp8(e4m3)
    DoubleRow pairs. sign(W) is exact in both dtypes; only x is rounded.
    fp8 DoubleRow matmuls process TWO k-tiles (256-deep contraction) per
    instruction at the same ~216ns as one bf16 k-tile (2x PE throughput;
    LDWEIGHTS fully hidden at FD=512). Per (out-block, token slice):
    16 bf16 + 8 DoubleRow matmuls = ~5.2us vs 6.9us all-bf16. Quantizing
    half of x to e4m3 gives deterministic rel err 1.88e-2 (vs 1.7e-3
    all-bf16), under the 2e-2 gate.
  - Each block runs its bf16 matmuls for all 4 token slices (kt-outer,
    tt-inner over 4 PSUM banks), then all DoubleRow matmuls — 2 PE
    perf-mode transitions per block instead of per token slice.
  - x.T is uploaded split: bf16 rows [0, 2048) and fp8 rows [2048, 4096),
    kept SBUF-resident (12 MB) as half-token tiles ([128, 1024] /
    [128, 2, 1024]) so phase 1 can run on the first token halves while
    the second halves stream in.
  - Phase 1 interleaves the first THREE out-blocks over two half-token
    passes (6 PSUM banks each) so the x-streaming prologue needs only
    ~230 GB/s and stays PE-bound; phase 2 runs the remaining 29 blocks
    against the resident x.
  - Weights are host-packed per out-block into bf16 [128, KB, 128] and
    fp8 [128, NP, 2, 128] blocks so every weight DMA is contiguous per
    partition row; phase-1 blocks stream their weights in k-chunks.
  - ScalarE evicts PSUM -> SBUF adding the bias (per-partition AP bias).
  - Output is written as out.T [4096, 2048] f32; host transposes back.

Throwaway warm-up matmuls on a memset tile run while the first DMAs are
in flight, flipping the PE's HAM clock gate to 2.4 GHz before the real
work starts.
"""

import numpy as np
import ml_dtypes

B = 8
T = 2048
IN_F = 4096
OUT_F = 4096
N_CORES = 8
P = 128
KT = IN_F // P  # 32 contraction tiles
OT = OUT_F // P  # 32 out-feature tiles
TN = 512  # moving-operand free dim (one PSUM bank of f32)
TT = T // TN  # 4 token slices
TH = T // 2  # half-token span (phase-1 pass granularity)

KB = 16  # bf16 k-tiles (k-tiles 0..KB-1)
KF = KT - KB  # fp8 k-tiles
NP = KF // 2  # fp8 DoubleRow pairs
NB1 = 4  # phase-1 interleaved out-blocks

_compiled_nc = None


def _chunks(n, sizes=(2, 2, 4, 8)):
    """Chunk 0..n into (offset, size) runs: small leading chunks keep the
    critical startup prefix small."""
    out = []
    off = 0
    i = 0
    while off < n:
        sz = min(sizes[i] if i < len(sizes) else sizes[-1], n - off)
        out.append((off, sz))
        off += sz
        i += 1
    return out


def build_program():
    import concourse.mybir as mybir
    import concourse.tile as tile
    from concourse import bacc

    DR = mybir.MatmulPerfMode.DoubleRow

    nc = bacc.Bacc("TRN2", target_bir_lowering=False, debug=False)

    xTb = nc.dram_tensor("xTb", [KB * P, T], mybir.dt.bfloat16, kind="ExternalInput")
    xTf = nc.dram_tensor("xTf", [KF * P, T], mybir.dt.float8e4, kind="ExternalInput")
    # Host-packed weights: wPb[ot, p, kt, o] = sign(W)[ot*128+o, kt*128+p],
    # wPf[ot, p, j, i, o] = sign(W)[ot*128+o, (KB+2j+i)*128+p].
    wPb = nc.dram_tensor("wPb", [OT, P, KB, P], mybir.dt.bfloat16, kind="ExternalInput")
    wPf = nc.dram_tensor(
        "wPf", [OT, P, NP, 2, P], mybir.dt.float8e4, kind="ExternalInput"
    )
    bv = nc.dram_tensor("biasv", [P, OT], mybir.dt.float32, kind="ExternalInput")
    oT = nc.dram_tensor("outT", [OUT_F, T], mybir.dt.float32, kind="ExternalOutput")

    xb_r = xTb.ap().rearrange("(kt p) t -> p kt t", p=P)  # [128, KB, 2048]
    xf_r = xTf.ap().rearrange("(kt p) t -> p kt t", p=P)  # [128, KF, 2048]
    oT_r = oT.ap().rearrange("(ot p) t -> p ot t", p=P)  # [128, 32, 2048]

    CHB = _chunks(KB)  # bf16 phase-1 weight chunks (in k-tiles)
    CHF = _chunks(NP, sizes=(4, 4))  # fp8 phase-1 weight chunks (in pairs)
    KT2CHB = {}
    for ci, (off, sz) in enumerate(CHB):
        for k in range(off, off + sz):
            KT2CHB[k] = (ci, off)
    J2CHF = {}
    for ci, (off, sz) in enumerate(CHF):
        for j in range(off, off + sz):
            J2CHF[j] = (ci, off)

    def evict(psum, ot, tt, lo=None, n=TN):
        if lo is None:
            lo = tt * TN
        o_sb = opool.tile([P, n], mybir.dt.float32, name=f"o_{ot}_{lo}", tag="o")
        nc.scalar.activation(
            o_sb[:],
            psum[:],
            mybir.ActivationFunctionType.Identity,
            bias=b_sb[:, ot : ot + 1],
        )
        nc.sync.dma_start(oT_r[:, ot, lo : lo + n], o_sb[:])

    with tile.TileContext(nc) as tc:
        with (
            tc.tile_pool(name="xpool", bufs=2 * KB + 2) as xpool,
            tc.tile_pool(name="xfpool", bufs=2 * NP) as xfpool,
            tc.tile_pool(name="wcbpool", bufs=NB1 * len(CHB)) as wcbpool,
            tc.tile_pool(name="wcfpool", bufs=NB1 * len(CHF)) as wcfpool,
            tc.tile_pool(name="wbpool", bufs=3) as wbpool,
            tc.tile_pool(name="wfpool", bufs=3) as wfpool,
            tc.tile_pool(name="bpool", bufs=2) as bpool,
            tc.tile_pool(name="opool", bufs=6) as opool,
            tc.tile_pool(name="pspool", bufs=8 * 512 // TN, space="PSUM") as pspool,
        ):
            # Warm up the PE while the first DMAs are in flight (HAM clock
            # gate -> 2.4 GHz).
            wu_x = bpool.tile([P, TN], mybir.dt.bfloat16, name="wu_x")
            nc.gpsimd.memset(wu_x[:], 0.0)
            wu_ps = pspool.tile([P, TN], mybir.dt.float32, name="wu_ps", tag="ps")
            for _ in range(12):
                nc.tensor.matmul(
                    wu_ps[:], wu_x[:, :P], wu_x[:], start=True, stop=True
                )

            # ---- phase-1 weight chunks (blocks 0..NB1-1) ----
            wcB = {}  # (b3, ci) -> bf16 chunk tile
            wcF = {}  # (b3, ci) -> fp8 chunk tile

            def load_chunk_b(b3, ci):
                off, sz = CHB[ci]
                w_t = wcbpool.tile(
                    [P, sz, P], mybir.dt.bfloat16, name=f"wcb_{b3}_{ci}", tag="wcb"
                )
                nc.sync.dma_start(w_t[:], wPb.ap()[b3][:, off : off + sz, :])
                wcB[(b3, ci)] = w_t

            def load_chunk_f(b3, ci):
                off, sz = CHF[ci]
                w_t = wcfpool.tile(
                    [P, sz, 2, P], mybir.dt.float8e4, name=f"wcf_{b3}_{ci}", tag="wcf"
                )
                nc.sync.dma_start(w_t[:], wPf.ap()[b3][:, off : off + sz, :, :])
                wcF[(b3, ci)] = w_t

            # ---- x tiles: kt=0 as four per-tt quarter tiles ([128, 512]);
            # kt>=1 as half-token tiles ([128, 1024]) so phase-1 pass h can
            # run on token half h while the other half streams ----
            x0q = {}  # tt -> bf16 tile [P, TN]
            xH = {}  # (kt, h) -> bf16 tile [P, TH]
            xfH = {}  # (j, h) -> fp8 pair tile [P, 2, TH]
            HN = TN // 2

            def load_x0q(tt):
                x_t = xpool.tile([P, TN], mybir.dt.bfloat16, name=f"x0_{tt}", tag="x")
                nc.sync.dma_start(x_t[:], xb_r[:, 0, tt * TN : (tt + 1) * TN])
                x0q[tt] = x_t

            def load_xh(kt, h):
                x_t = xpool.tile(
                    [P, TH], mybir.dt.bfloat16, name=f"x_{kt}_{h}", tag="x"
                )
                nc.sync.dma_start(x_t[:], xb_r[:, kt, h * TH : (h + 1) * TH])
                xH[(kt, h)] = x_t

            def load_xfh(j, h):
                x_t = xfpool.tile(
                    [P, 2, TH], mybir.dt.float8e4, name=f"xf_{j}_{h}", tag="xf"
                )
                nc.sync.dma_start(
                    x_t[:], xf_r[:, 2 * j : 2 * j + 2, h * TH : (h + 1) * TH]
                )
                xfH[(j, h)] = x_t

            def x_slice(kt, tt, lo=0, n=TN):
                if kt == 0:
                    return x0q[tt][:, lo : lo + n]
                tl = (tt % 2) * TN + lo
                return xH[(kt, tt // 2)][:, tl : tl + n]

            def xf_slice(j, tt, lo=0, n=TN):
                tl = (tt % 2) * TN + lo
                return xfH[(j, tt // 2)][:, :, tl : tl + n]

            # ---- DMA issue order: by first-use time ----
            load_chunk_b(0, 0)
            load_x0q(0)
            load_chunk_b(1, 0)
            load_x0q(1)
            load_chunk_b(2, 0)
            load_chunk_b(3, 0)
            load_xh(1, 0)
            for b3 in range(NB1):
                load_chunk_b(b3, 1)
            load_xh(2, 0)
            load_xh(3, 0)
            for b3 in range(NB1):
                load_chunk_b(b3, 2)
            for kt in range(4, 8):
                load_xh(kt, 0)
            for b3 in range(NB1):
                load_chunk_b(b3, 3)
            for kt in range(8, KB):
                load_xh(kt, 0)
            for b3 in range(NB1):
                load_chunk_f(b3, 0)
            for j in range(0, 4):
                load_xfh(j, 0)
            for b3 in range(NB1):
                for ci in range(1, len(CHF)):
                    load_chunk_f(b3, ci)
            for j in range(4, NP):
                load_xfh(j, 0)
            # Bias is tiny but descriptor-heavy; first needed at the first
            # eviction (~45us in).
            b_sb = bpool.tile([P, OT], mybir.dt.float32, name="b_sb")
            nc.sync.dma_start(b_sb[:], bv.ap())
            # second token half
            load_x0q(2)
            load_x0q(3)
            for kt in range(1, KB):
                load_xh(kt, 1)
            for j in range(NP):
                load_xfh(j, 1)

            # ---- phase 1: blocks 0..NB1-1, one pass per token half
            # (NB1 * 2 = 8 PSUM banks per pass) ----
            for h in range(2):
                tts = (2 * h, 2 * h + 1)
                ps1 = {
                    (b3, tt): pspool.tile(
                        [P, TN], mybir.dt.float32, name=f"ps1_{b3}_{tt}", tag="ps"
                    )
                    for b3 in range(NB1)
                    for tt in tts
                }
                for kt in range(KB):
                    ci, off = KT2CHB[kt]
                    for tt in tts:
                        for b3 in range(NB1):
                            lhsT = wcB[(b3, ci)][:, kt - off, :]
                            nc.tensor.matmul(
                                ps1[(b3, tt)][:],
                                lhsT,
                                x_slice(kt, tt),
                                start=(kt == 0),
                                stop=False,
                            )
                for j in range(NP):
                    ci, off = J2CHF[j]
                    for tt in tts:
                        for b3 in range(NB1):
                            lhsT = wcF[(b3, ci)][:, j - off, :, :]
                            nc.tensor.matmul(
                                ps1[(b3, tt)][:],
                                lhsT,
                                xf_slice(j, tt),
                                start=False,
                                stop=(j == NP - 1),
                                perf_mode=DR,
                            )
                for b3 in range(NB1):
                    for tt in tts:
                        evict(ps1[(b3, tt)], b3, tt)

            # ---- phase 2: remaining blocks against the resident x ----
            for ot in range(NB1, OT):
                wb_sb = wbpool.tile(
                    [P, KB, P], mybir.dt.bfloat16, name=f"wb_{ot}", tag="wb"
                )
                nc.sync.dma_start(wb_sb[:], wPb.ap()[ot])
                wf_sb = wfpool.tile(
                    [P, NP, 2, P], mybir.dt.float8e4, name=f"wf_{ot}", tag="wf"
                )
                nc.sync.dma_start(wf_sb[:], wPf.ap()[ot])

                last_block = ot == OT - 1
                n_tt = TT - 1 if last_block else TT
                psums = [
                    pspool.tile(
                        [P, TN], mybir.dt.float32, name=f"ps_{ot}_{tt}", tag="ps"
                    )
                    for tt in range(n_tt)
                ]
                # all bf16 k-tiles (kt-outer, tt-inner), then all DoubleRow
                # pairs: 2 PE perf-mode transitions per block, and each
                # PSUM bank still finishes within ~2us of the block's end.
                for kt in range(KB):
                    lhsT = wb_sb[:, kt, :]
                    for tt in range(n_tt):
                        nc.tensor.matmul(
                            psums[tt][:],
                            lhsT,
                            x_slice(kt, tt),
                            start=(kt == 0),
                            stop=False,
                        )
                for j in range(NP):
                    lhsT = wf_sb[:, j, :, :]
                    for tt in range(n_tt):
                        nc.tensor.matmul(
                            psums[tt][:],
                            lhsT,
                            xf_slice(j, tt),
                            start=False,
                            stop=(j == NP - 1),
                            perf_mode=DR,
                        )
                for tt in range(n_tt):
                    evict(psums[tt], ot, tt)

                if last_block:
                    # The kernel's very last group (tt=3) is split into two
                    # half-width groups run sequentially, so the final
                    # evict+DMA chain (which nothing can overlap) covers
                    # 128KB instead of 256KB.
                    for h in range(2):
                        psum = pspool.tile(
                            [P, HN], mybir.dt.float32, name=f"ps_l_{h}", tag="ps"
                        )
                        lo = 3 * TN + h * HN
                        for kt in range(KB):
                            nc.tensor.matmul(
                                psum[:],
                                wb_sb[:, kt, :],
                                x_slice(kt, 3, lo=h * HN, n=HN),
                                start=(kt == 0),
                                stop=False,
                            )
                        for j in range(NP):
                            nc.tensor.matmul(
                                psum[:],
                                wf_sb[:, j, :, :],
                                xf_slice(j, 3, lo=h * HN, n=HN),
                                start=False,
                                stop=(j == NP - 1),
                                perf_mode=DR,
                            )
                        evict(psum, ot, None, lo=lo, n=HN)

    nc.compile()
    return nc


def prepare_inputs(x, weight, bias):
    """Host-side layout prep: transpose + cast per-core shards."""
    bf16 = ml_dtypes.bfloat16
    fp8 = ml_dtypes.float8_e4m3
    x = np.asarray(x, dtype=np.float32)
    weight = np.asarray(weight, dtype=np.float32)
    bias = np.asarray(bias, dtype=np.float32)
    w_bin = np.where(weight >= 0, np.float32(1.0), np.float32(-1.0))
    # wP[ot, p, kt, o] = sign(W)[ot*128+o, kt*128+p] — per-ot weight blocks,
    # contiguous along (kt, o) so block DMAs are contiguous per partition.
    wP_np = np.ascontiguousarray(w_bin.reshape(OT, P, KT, P).transpose(0, 3, 2, 1))
    wPb_np = np.ascontiguousarray(wP_np[:, :, :KB, :]).astype(bf16)
    wPf_np = np.ascontiguousarray(wP_np[:, :, KB:, :]).astype(fp8).reshape(
        OT, P, NP, 2, P
    )
    bv_np = np.ascontiguousarray(
        np.asarray(bias, dtype=np.float32).reshape(OT, P).T
    )  # [P, OT]; bias[o] at [o % 128, o // 128]
    in_maps = []
    for b in range(B):
        xT_np = np.ascontiguousarray(x[b].T)  # [in, tokens]
        in_maps.append(
            {
                "xTb": xT_np[: KB * P].astype(bf16),
                "xTf": xT_np[KB * P :].astype(fp8),
                "wPb": wPb_np,
                "wPf": wPf_np,
                "biasv": bv_np,
            }
        )
    return in_maps


def _ensure_ntff_hook_shim():
    """bass_utils' trace path imports antenv.axon_hooks, which some images
    lack; provide a working shim (or a None hook) so tracing never crashes."""
    import sys
    import types

    try:
        import antenv.axon_hooks  # noqa: F401

        return
    except ImportError:
        pass
    hook = None
    try:
        from trn_agent_boot.trn_boot import _ntff_profile_via_ctypes

        hook = _ntff_profile_via_ctypes("/opt/axon/libaxon_pjrt.so")
    except Exception:
        pass
    mod = types.ModuleType("antenv.axon_hooks")
    mod.get_axon_ntff_profile_hook = lambda: hook
    mod.set_axon_ntff_profile_hook = lambda h: None
    sys.modules["antenv.axon_hooks"] = mod
    try:
        import antenv

        antenv.axon_hooks = mod
    except ImportError:
        pass


def run(in_maps, trace=False, **kwargs):
    global _compiled_nc
    if _compiled_nc is None:
        _compiled_nc = build_program()
    _ensure_ntff_hook_shim()
    from concourse.bass_utils import run_bass_kernel_spmd

    return run_bass_kernel_spmd(
        _compiled_nc, in_maps, list(range(N_CORES)), trace=trace, **kwargs
    )


def kernel(x, weight, bias):
    res = run(prepare_inputs(x, weight, bias))
    out = np.empty((B, T, OUT_F), dtype=np.float32)
    for b in range(B):
        out[b] = res.results[b]["outT"].T
    return out


# revision 13
# speedup vs baseline: 1.1955x; 1.1955x over previous
"""BinaryLinear kernel for 8 Trainium2 NeuronCores.

Computes out = x @ sign(W).T + bias for x [8, 2048, 4096], W [4096, 4096],
bias [4096], all float32.

Strategy: data-parallel over the batch dim — core b handles x[b] ([2048
tokens, 4096 in]) with the full (binarized) weight matrix.

Per-core device kernel (Tile framework) — MIXED-PRECISION contraction:
  - The 32 contraction k-tiles are split KB=16 in bf16 + 8 fp8(e4m3)
    DoubleRow pairs. sign(W) is exact in both dtypes; only x is rounded.
    fp8 DoubleRow matmuls process TWO k-tiles (256-deep contraction) per
    instruction at the same ~216ns as one bf16 k-tile (2x PE throughput;
    LDWEIGHTS fully hidden at FD=512). Per (out-block, token slice):
    16 bf16 + 8 DoubleRow matmuls = ~5.2us vs 6.9us all-bf16. Quantizing
    half of x to e4m3 gives deterministic rel err 1.88e-2 (vs 1.7e-3
    all-bf16), under the 2e-2 gate.
  - Each block runs its bf16 matmuls for all 4 token slices (kt-outer,
    tt-inner over 4 PSUM banks), then all DoubleRow matmuls — 2 PE
    perf-mode transitions per block instead of per token slice.
  - x.T is uploaded split: bf16 rows [0, 2048) and fp8 rows [2048, 4096),
    kept SBUF-resident (12 MB) as half-token tiles ([128, 1024] /
    [128, 2, 1024]) so phase 1 can run on the first token halves while
    the second halves stream in.
  - Phase 1 interleaves the first THREE out-blocks over two half-token
    passes (6 PSUM banks each) so the x-streaming prologue needs only
    ~230 GB/s and stays PE-bound; phase 2 runs the remaining 29 blocks
    against the resident x.
  - Weights are host-packed per out-block into bf16 [128, KB, 128] and
    fp8 [128, NP, 2, 128] blocks so every weight DMA is contiguous per
    partition row; phase-1 blocks stream their weights in k-chunks.
  - ScalarE evicts PSUM -> SBUF adding the bias (per-partition AP bias).
  - Output is written as out.T [4096, 2048] f32; host transposes back.

Throwaway warm-up matmuls on a memset tile run while the first DMAs are
in flight, flipping the PE's HAM clock gate to 2.4 GHz before the real
work starts.
"""

import numpy as np
import ml_dtypes

B = 8
T = 2048
IN_F = 4096
OUT_F = 4096
N_CORES = 8
P = 128
KT = IN_F // P  # 32 contraction tiles
OT = OUT_F // P  # 32 out-feature tiles
TN = 512  # moving-operand free dim (one PSUM bank of f32)
TT = T // TN  # 4 token slices
TH = T // 2  # half-token span (phase-1 pass granularity)

KB = 16  # bf16 k-tiles (k-tiles 0..KB-1)
KF = KT - KB  # fp8 k-tiles
NP = KF // 2  # fp8 DoubleRow pairs
NB1 = 4  # phase-1 interleaved out-blocks

_compiled_nc = None


def _chunks(n, sizes=(2, 2, 4, 8)):
    """Chunk 0..n into (offset, size) runs: small leading chunks keep the
    critical startup prefix small."""
    out = []
    off = 0
    i = 0
    while off < n:
        sz = min(sizes[i] if i < len(sizes) else sizes[-1], n - off)
        out.append((off, sz))
        off += sz
        i += 1
    return out


def build_program():
    import concourse.mybir as mybir
    import concourse.tile as tile
    from concourse import bacc

    DR = mybir.MatmulPerfMode.DoubleRow

    nc = bacc.Bacc("TRN2", target_bir_lowering=False, debug=False)

    xTb = nc.dram_tensor("xTb", [KB * P, T], mybir.dt.bfloat16, kind="ExternalInput")
    xTf = nc.dram_tensor("xTf", [KF * P, T], mybir.dt.float8e4, kind="ExternalInput")
    # Host-packed weights: wPb[ot, p, kt, o] = sign(W)[ot*128+o, kt*128+p],
    # wPf[ot, p, j, i, o] = sign(W)[ot*128+o, (KB+2j+i)*128+p].
    wPb = nc.dram_tensor("wPb", [OT, P, KB, P], mybir.dt.bfloat16, kind="ExternalInput")
    wPf = nc.dram_tensor(
        "wPf", [OT, P, NP, 2, P], mybir.dt.float8e4, kind="ExternalInput"
    )
    bv = nc.dram_tensor("biasv", [P, OT], mybir.dt.float32, kind="ExternalInput")
    oT = nc.dram_tensor("outT", [OUT_F, T], mybir.dt.float32, kind="ExternalOutput")

    xb_r = xTb.ap().rearrange("(kt p) t -> p kt t", p=P)  # [128, KB, 2048]
    xf_r = xTf.ap().rearrange("(kt p) t -> p kt t", p=P)  # [128, KF, 2048]
    oT_r = oT.ap().rearrange("(ot p) t -> p ot t", p=P)  # [128, 32, 2048]

    CHB = _chunks(KB)  # bf16 phase-1 weight chunks (in k-tiles)
    CHF = _chunks(NP, sizes=(4, 4))  # fp8 phase-1 weight chunks (in pairs)
    KT2CHB = {}
    for ci, (off, sz) in enumerate(CHB):
        for k in range(off, off + sz):
            KT2CHB[k] = (ci, off)
    J2CHF = {}
    for ci, (off, sz) in enumerate(CHF):
        for j in range(off, off + sz):
            J2CHF[j] = (ci, off)

    def evict(psum, ot, tt, lo=None, n=TN):
        if lo is None:
            lo = tt * TN
        o_sb = opool.tile([P, n], mybir.dt.float32, name=f"o_{ot}_{lo}", tag="o")
        nc.scalar.activation(
            o_sb[:],
            psum[:],
            mybir.ActivationFunctionType.Identity,
            bias=b_sb[:, ot : ot + 1],
        )
        nc.sync.dma_start(oT_r[:, ot, lo : lo + n], o_sb[:])

    with tile.TileContext(nc) as tc:
        with (
            tc.tile_pool(name="xpool", bufs=2 * KB + 2) as xpool,
            tc.tile_pool(name="xfpool", bufs=2 * NP) as xfpool,
            tc.tile_pool(name="wcbpool", bufs=NB1 * len(CHB)) as wcbpool,
            tc.tile_pool(name="wcfpool", bufs=NB1 * len(CHF)) as wcfpool,
            tc.tile_pool(name="wbpool", bufs=3) as wbpool,
            tc.tile_pool(name="wfpool", bufs=3) as wfpool,
            tc.tile_pool(name="bpool", bufs=2) as bpool,
            tc.tile_pool(name="opool", bufs=6) as opool,
            tc.tile_pool(name="pspool", bufs=8 * 512 // TN, space="PSUM") as pspool,
        ):
            # Warm up the PE while the first DMAs are in flight (HAM clock
            # gate -> 2.4 GHz).
            wu_x = bpool.tile([P, TN], mybir.dt.bfloat16, name="wu_x")
            nc.gpsimd.memset(wu_x[:], 0.0)
            wu_ps = pspool.tile([P, TN], mybir.dt.float32, name="wu_ps", tag="ps")
            for _ in range(12):
                nc.tensor.matmul(
                    wu_ps[:], wu_x[:, :P], wu_x[:], start=True, stop=True
                )

            # ---- phase-1 weight chunks (blocks 0..NB1-1) ----
            wcB = {}  # (b3, ci) -> bf16 chunk tile
            wcF = {}  # (b3, ci) -> fp8 chunk tile

            def load_chunk_b(b3, ci):
                off, sz = CHB[ci]
                w_t = wcbpool.tile(
                    [P, sz, P], mybir.dt.bfloat16, name=f"wcb_{b3}_{ci}", tag="wcb"
                )
                nc.sync.dma_start(w_t[:], wPb.ap()[b3][:, off : off + sz, :])
                wcB[(b3, ci)] = w_t

            def load_chunk_f(b3, ci):
                off, sz = CHF[ci]
                w_t = wcfpool.tile(
                    [P, sz, 2, P], mybir.dt.float8e4, name=f"wcf_{b3}_{ci}", tag="wcf"
                )
                nc.sync.dma_start(w_t[:], wPf.ap()[b3][:, off : off + sz, :, :])
                wcF[(b3, ci)] = w_t

            # ---- x tiles: kt=0 as four per-tt quarter tiles ([128, 512]);
            # kt>=1 as half-token tiles ([128, 1024]) so phase-1 pass h can
            # run on token half h while the other half streams ----
            x0q = {}  # tt -> bf16 tile [P, TN]
            xH = {}  # (kt, h) -> bf16 tile [P, TH]
            xfH = {}  # (j, h) -> fp8 pair tile [P, 2, TH]
            HN = TN // 2

            def load_x0q(tt):
                x_t = xpool.tile([P, TN], mybir.dt.bfloat16, name=f"x0_{tt}", tag="x")
                nc.sync.dma_start(x_t[:], xb_r[:, 0, tt * TN : (tt + 1) * TN])
                x0q[tt] = x_t

            def load_xh(kt, h):
                x_t = xpool.tile(
                    [P, TH], mybir.dt.bfloat16, name=f"x_{kt}_{h}", tag="x"
                )
                nc.sync.dma_start(x_t[:], xb_r[:, kt, h * TH : (h + 1) * TH])
                xH[(kt, h)] = x_t

            def load_xfh(j, h):
                x_t = xfpool.tile(
                    [P, 2, TH], mybir.dt.float8e4, name=f"xf_{j}_{h}", tag="xf"
                )
                nc.sync.dma_start(
                    x_t[:], xf_r[:, 2 * j : 2 * j + 2, h * TH : (h + 1) * TH]
                )
                xfH[(j, h)] = x_t

            def x_slice(kt, tt, lo=0, n=TN):
                if kt == 0:
                    return x0q[tt][:, lo : lo + n]
                tl = (tt % 2) * TN + lo
                return xH[(kt, tt // 2)][:, tl : tl + n]

            def xf_slice(j, tt, lo=0, n=TN):
                tl = (tt % 2) * TN + lo
                return xfH[(j, tt // 2)][:, :, tl : tl + n]

            # ---- DMA issue order: by first-use time ----
            load_chunk_b(0, 0)
            load_x0q(0)
            load_chunk_b(1, 0)
            load_chunk_b(2, 0)
            load_chunk_b(3, 0)
            load_x0q(1)
            load_xh(1, 0)
            for b3 in range(NB1):
                load_chunk_b(b3, 1)
            load_xh(2, 0)
            load_xh(3, 0)
            for b3 in range(NB1):
                load_chunk_b(b3, 2)
            for kt in range(4, 8):
                load_xh(kt, 0)
            for b3 in range(NB1):
                load_chunk_b(b3, 3)
            for kt in range(8, KB):
                load_xh(kt, 0)
            for b3 in range(NB1):
                load_chunk_f(b3, 0)
            for j in range(0, 4):
                load_xfh(j, 0)
            for b3 in range(NB1):
                for ci in range(1, len(CHF)):
                    load_chunk_f(b3, ci)
            for j in range(4, NP):
                load_xfh(j, 0)
            # Bias is tiny but descriptor-heavy; first needed at the first
            # eviction (~45us in).
            b_sb = bpool.tile([P, OT], mybir.dt.float32, name="b_sb")
            nc.sync.dma_start(b_sb[:], bv.ap())
            # second token half
            load_x0q(2)
            load_x0q(3)
            for kt in range(1, KB):
                load_xh(kt, 1)
            for j in range(NP):
                load_xfh(j, 1)

            # ---- phase 1: blocks 0..NB1-1, one pass per token half
            # (NB1 * 2 = 8 PSUM banks per pass) ----
            for h in range(2):
                tts = (2 * h, 2 * h + 1)
                ps1 = {
                    (b3, tt): pspool.tile(
                        [P, TN], mybir.dt.float32, name=f"ps1_{b3}_{tt}", tag="ps"
                    )
                    for b3 in range(NB1)
                    for tt in tts
                }
                for kt in range(KB):
                    ci, off = KT2CHB[kt]
                    for tt in tts:
                        for b3 in range(NB1):
                            lhsT = wcB[(b3, ci)][:, kt - off, :]
                            nc.tensor.matmul(
                                ps1[(b3, tt)][:],
                                lhsT,
                                x_slice(kt, tt),
                                start=(kt == 0),
                                stop=False,
                            )
                for j in range(NP):
                    ci, off = J2CHF[j]
                    for tt in tts:
                        for b3 in range(NB1):
                            lhsT = wcF[(b3, ci)][:, j - off, :, :]
                            nc.tensor.matmul(
                                ps1[(b3, tt)][:],
                                lhsT,
                                xf_slice(j, tt),
                                start=False,
                                stop=(j == NP - 1),
                                perf_mode=DR,
                            )
                for b3 in range(NB1):
                    for tt in tts:
                        evict(ps1[(b3, tt)], b3, tt)

            # ---- phase 2: remaining blocks against the resident x ----
            for ot in range(NB1, OT):
                wb_sb = wbpool.tile(
                    [P, KB, P], mybir.dt.bfloat16, name=f"wb_{ot}", tag="wb"
                )
                nc.sync.dma_start(wb_sb[:], wPb.ap()[ot])
                wf_sb = wfpool.tile(
                    [P, NP, 2, P], mybir.dt.float8e4, name=f"wf_{ot}", tag="wf"
                )
                nc.sync.dma_start(wf_sb[:], wPf.ap()[ot])

                last_block = ot == OT - 1
                n_tt = TT - 1 if last_block else TT
                psums = [
                    pspool.tile(
                        [P, TN], mybir.dt.float32, name=f"ps_{ot}_{tt}", tag="ps"
                    )
                    for tt in range(n_tt)
                ]
                # all bf16 k-tiles (kt-outer, tt-inner), then all DoubleRow
                # pairs: 2 PE perf-mode transitions per block, and each
                # PSUM bank still finishes within ~2us of the block's end.
                for kt in range(KB):
                    lhsT = wb_sb[:, kt, :]
                    for tt in range(n_tt):
                        nc.tensor.matmul(
                            psums[tt][:],
                            lhsT,
                            x_slice(kt, tt),
                            start=(kt == 0),
                            stop=False,
                        )
                for j in range(NP):
                    lhsT = wf_sb[:, j, :, :]
                    for tt in range(n_tt):
                        nc.tensor.matmul(
                            psums[tt][:],
                            lhsT,
                            xf_slice(j, tt),
                            start=False,
                            stop=(j == NP - 1),
                            perf_mode=DR,
                        )
                for tt in range(n_tt):
                    evict(psums[tt], ot, tt)

                if last_block:
                    # The kernel's very last group (tt=3) is split into two
                    # half-width groups run sequentially, so the final
                    # evict+DMA chain (which nothing can overlap) covers
                    # 128KB instead of 256KB.
                    for h in range(2):
                        psum = pspool.tile(
                            [P, HN], mybir.dt.float32, name=f"ps_l_{h}", tag="ps"
                        )
                        lo = 3 * TN + h * HN
                        for kt in range(KB):
                            nc.tensor.matmul(
                                psum[:],
                                wb_sb[:, kt, :],
                                x_slice(kt, 3, lo=h * HN, n=HN),
                                start=(kt == 0),
                                stop=False,
                            )
                        for j in range(NP):
                            nc.tensor.matmul(
                                psum[:],
                                wf_sb[:, j, :, :],
                                xf_slice(j, 3, lo=h * HN, n=HN),
                                start=False,
                                stop=(j == NP - 1),
                                perf_mode=DR,
                            )
                        evict(psum, ot, None, lo=lo, n=HN)

    nc.compile()
    return nc


def prepare_inputs(x, weight, bias):
    """Host-side layout prep: transpose + cast per-core shards."""
    bf16 = ml_dtypes.bfloat16
    fp8 = ml_dtypes.float8_e4m3
    x = np.asarray(x, dtype=np.float32)
    weight = np.asarray(weight, dtype=np.float32)
    bias = np.asarray(bias, dtype=np.float32)
    w_bin = np.where(weight >= 0, np.float32(1.0), np.float32(-1.0))
    # wP[ot, p, kt, o] = sign(W)[ot*128+o, kt*128+p] — per-ot weight blocks,
    # contiguous along (kt, o) so block DMAs are contiguous per partition.
    wP_np = np.ascontiguousarray(w_bin.reshape(OT, P, KT, P).transpose(0, 3, 2, 1))
    wPb_np = np.ascontiguousarray(wP_np[:, :, :KB, :]).astype(bf16)
    wPf_np = np.ascontiguousarray(wP_np[:, :, KB:, :]).astype(fp8).reshape(
        OT, P, NP, 2, P
    )
    bv_np = np.ascontiguousarray(
        np.asarray(bias, dtype=np.float32).reshape(OT, P).T
    )  # [P, OT]; bias[o] at [o % 128, o // 128]
    in_maps = []
    for b in range(B):
        xT_np = np.ascontiguousarray(x[b].T)  # [in, tokens]
        in_maps.append(
            {
                "xTb": xT_np[: KB * P].astype(bf16),
                "xTf": xT_np[KB * P :].astype(fp8),
                "wPb": wPb_np,
                "wPf": wPf_np,
                "biasv": bv_np,
            }
        )
    return in_maps


def _ensure_ntff_hook_shim():
    """bass_utils' trace path imports antenv.axon_hooks, which some images
    lack; provide a working shim (or a None hook) so tracing never crashes."""
    import sys
    import types

    try:
        import antenv.axon_hooks  # noqa: F401

        return
    except ImportError:
        pass
    hook = None
    try:
        from trn_agent_boot.trn_boot import _ntff_profile_via_ctypes

        hook = _ntff_profile_via_ctypes("/opt/axon/libaxon_pjrt.so")
    except Exception:
        pass
    mod = types.ModuleType("antenv.axon_hooks")
    mod.get_axon_ntff_profile_hook = lambda: hook
    mod.set_axon_ntff_profile_hook = lambda h: None
    sys.modules["antenv.axon_hooks"] = mod
    try:
        import antenv

        antenv.axon_hooks = mod
    except ImportError:
        pass


def run(in_maps, trace=False, **kwargs):
    global _compiled_nc
    if _compiled_nc is None:
        _compiled_nc = build_program()
    _ensure_ntff_hook_shim()
    from concourse.bass_utils import run_bass_kernel_spmd

    return run_bass_kernel_spmd(
        _compiled_nc, in_maps, list(range(N_CORES)), trace=trace, **kwargs
    )


def kernel(x, weight, bias):
    res = run(prepare_inputs(x, weight, bias))
    out = np.empty((B, T, OUT_F), dtype=np.float32)
    for b in range(B):
        out[b] = res.results[b]["outT"].T
    return out
